# revision 4
# baseline (speedup 1.0000x reference)
"""Chebyshev graph-conv kernel for Trainium2 (8 NeuronCores, SPMD).

Math: out[b,o,m,t] = sum_{k,c,n} T[k,n,m] * x[b,c,n,t] * Theta[k,c,o]
with T the Chebyshev polynomials of the normalized adjacency (n=24, K=3).

The whole operator collapses into a single 768x768 matrix
    W[(c,n),(o,m)] = sum_k Theta[k,c,o] * T[k,n,m]
applied per batch element to x[b] viewed as (c*n, t) = (768, 512):
    out[b](o*24+m, t) = W.T-contract over rows -> exactly one matmul chain.

W is tiny and computed on host from adj/Theta; x is read once and out
written once. Data-parallel over batch: 64 -> 8 per core. All device I/O is
fp16 (PE multiplies 16-bit at full rate with hidden weight loads; fp16 I/O
halves HBM traffic both ways). PSUM accumulation is fp32; the final fp32
cast happens on host (fp16 store rounding costs ~2e-4 extra relative
error). Per core: 8 batch elements, each a 6x6 chain of [128,128]x[128,512]
matmuls.

Schedule notes (from NTFF traces): the bass framework preamble costs a
fixed ~7.2us before any kernel instruction, and the PE-bound matmul stream
(288 MMs @ 215.5ns warm) is the roofline at ~62us. So the kernel is
structured to start real matmuls as early as possible after the preamble:

- i-outer loop order: all 6 output-chunk PSUM banks accumulate in parallel
  per contraction chunk i, so the first matmul needs only W[0]/x0[0]
  (0.3 MB) instead of the whole first batch (1.95 MB).
- W chunks go on the Sync HWDGE ring while x0 chunks go concurrently on
  the Scalar ring (stores don't exist yet), halving time-to-first-chunk.
- HAM warm-up: the PE boots clock-gated at K=4/8 (1.2 GHz) and unthrottles
  only after ~3.4us of sustained busy. A vector-engine memset (no slow
  gpsimd dependency) plus a few dummy matmuls keep the PE busy from the
  first possible instant so the unthrottle deadline starts ticking ~7.4us
  rather than when DMA data lands.
"""

import numpy as np

import concourse.mybir as mybir
from concourse import bacc, tile
from concourse.bass_utils import run_bass_kernel_spmd

N_CORES = 8
B, C, NV, T = 64, 32, 24, 512
K = 3
O = 32
CN = C * NV   # 768 contraction rows
OM = O * NV   # 768 output rows
BP = B // N_CORES  # 8 batch elements per core
P = 128
NBLK = CN // P  # 6

_compiled_nc = None
last_result = None  # BassKernelResults from the most recent run (for test.py)


def _build_nc():
    f32 = mybir.dt.float32
    f16 = mybir.dt.float16
    nc = bacc.Bacc("TRN2", target_bir_lowering=False, debug=False,
                   num_devices=N_CORES)
    xs = nc.dram_tensor("xs", [BP, CN, T], f16, kind="ExternalInput")
    w = nc.dram_tensor("w", [CN, OM], f16, kind="ExternalInput")
    out = nc.dram_tensor("out", [BP, OM, T], f16, kind="ExternalOutput")

    wr = w[:].rearrange("(i p) m -> p i m", p=P)

    with tile.TileContext(nc) as tc:
        with (
            tc.tile_pool(name="wpool", bufs=1) as wpool,
            tc.tile_pool(name="xpool", bufs=8) as xpool,
            tc.tile_pool(name="opool", bufs=6) as opool,
            tc.tile_pool(name="psum", bufs=8, space="PSUM") as psum_pool,
        ):
            # HAM warm-up: memset on the vector engine (fast, no gpsimd
            # spin-up) then dummy matmuls on the zeroed tile. 6 x 256-col
            # cold MMs ~= 1.3us of PE busy, bridging the gap until the
            # first real operands land without delaying them.
            warm = wpool.tile([P, T], f16, tag="warm")
            nc.vector.memset(warm[:], 0.0)
            for _ in range(6):
                wps = psum_pool.tile([P, T], f32, tag="ps")
                nc.tensor.matmul(wps[:, :256], warm[:, :P], warm[:, :256],
                                 start=True, stop=True)

            # W chunks on the Sync ring, x0 chunks concurrently on the
            # Scalar ring (stores only start much later, so no head-of-line
            # blocking). First matmul needs just wt[:,0,:] + xt0[:,0,:].
            wt = wpool.tile([P, NBLK, OM], f16)
            xt0 = xpool.tile([P, NBLK, T], f16)
            xr0 = xs[0].rearrange("(i p) t -> p i t", p=P)
            for i in range(NBLK):
                nc.sync.dma_start(wt[:, i, :], wr[:, i, :])
                nc.scalar.dma_start(xt0[:, i, :], xr0[:, i, :])

            xts = [xt0]
            for b in range(1, BP):
                xt = xpool.tile([P, NBLK, T], f16, tag="xt0")
                xr = xs[b].rearrange("(i p) t -> p i t", p=P)
                nc.sync.dma_start(xt[:], xr)
                xts.append(xt)

            for b in range(BP):
                xt = xts[b]
                ot = opool.tile([P, NBLK, T], f16)
                orr = out[b].rearrange("(j p) t -> p j t", p=P)
                # i-outer: the 6 j-banks accumulate in parallel so chunk i
                # is fully consumed by 6 back-to-back matmuls as soon as it
                # arrives; batch 0 never waits for more than one chunk.
                pss = [psum_pool.tile([P, T], f32, name=f"ps_b{b}_j{j}",
                                      tag="ps")
                       for j in range(NBLK)]
                for i in range(NBLK):
                    for j in range(NBLK):
                        nc.tensor.matmul(
                            pss[j][:],
                            wt[:, i, j * P:(j + 1) * P],
                            xt[:, i, :],
                            start=(i == 0),
                            stop=(i == NBLK - 1),
                        )
                for j in range(NBLK):
                    # fp32 PSUM -> fp16 SBUF cast on the vector engine
                    # (16-bit output runs at 2x DVE rate), then store on the
                    # Scalar ring; stores are production-paced and the fp16
                    # halving keeps them off the loads' critical path.
                    nc.vector.tensor_copy(ot[:, j, :], pss[j][:])
                    nc.scalar.dma_start(orr[:, j, :], ot[:, j, :])

    nc.compile()
    return nc


def _combined_operator(adj: np.ndarray, Theta: np.ndarray) -> np.ndarray:
    """W[(c,n),(o,m)] = sum_k Theta[k,c,o] * T[k,n,m], fp16, shape (768,768)."""
    adj = np.asarray(adj).astype(np.float32)
    Theta = np.asarray(Theta)
    d = adj.sum(axis=1)
    d_inv_sqrt = np.where(d > 0, 1.0 / np.sqrt(d), 0.0).astype(np.float32)
    L = (adj * d_inv_sqrt[None, :]).T * d_inv_sqrt[None, :]
    Ts = [np.eye(NV, dtype=np.float32), L.astype(np.float32)]
    for _ in range(2, K):
        Ts.append((2.0 * L @ Ts[-1] - Ts[-2]).astype(np.float32))
    Tcheb = np.stack(Ts[:K])  # (K, n, m)
    W = np.einsum("kco,knm->cnom", Theta.astype(np.float32), Tcheb)
    return np.ascontiguousarray(W.reshape(CN, OM), dtype=np.float16)


def kernel(x: np.ndarray, adj: np.ndarray, Theta: np.ndarray) -> np.ndarray:
    global _compiled_nc, last_result
    if _compiled_nc is None:
        _compiled_nc = _build_nc()
    nc = _compiled_nc

    W = _combined_operator(adj, Theta)
    # x: (64, 32, 24, 512) -> per-core shard [8, 768, 512], fp16 (the device
    # matmul consumes fp16 regardless; casting host-side halves HBM reads)
    xf = np.asarray(x).astype(np.float16).reshape(B, CN, T)
    in_maps = [
        {"xs": np.ascontiguousarray(xf[c * BP:(c + 1) * BP]), "w": W}
        for c in range(N_CORES)
    ]
    res = run_bass_kernel_spmd(nc, in_maps, core_ids=list(range(N_CORES)))
    last_result = res
    out = np.concatenate([r["out"] for r in res.results], axis=0)
    return np.ascontiguousarray(out.reshape(B, O, NV, T).astype(np.float32))
